# revision 5
# baseline (speedup 1.0000x reference)
"""Trainium2 Bass kernel for nn_DeepFM_3066606649824.

Strategy (8 NeuronCores, data-parallel over batch; SHARD = 512 rows/core):
  - Host: restructure the 26 FFM tables [26, 208000, 16] f32 into one bf16
    row-major table G2 [208000, 512]: cols 0:416 = the 26 tables' rows for
    that index (feature f = 16*i + d), col 416 = fm1_emb, rest zero.
    1024B rows satisfy dma_gather's 256B-multiple elem/stride rule.
  - Gather via InstDMAGatherAnt (Q7 'mlp' library): 13 HBM gathers of 1024
    rows (2 fields x 512 batch) land g_all [128, 104rows, 512] (row 4j+s),
    then 26 SBUF-source TRANSPOSED dma_gathers (512 idxs each, <=512 per
    instr is a Q7 limit; HBM gathers cap at 1024) produce gT tiles
    [128, 4, 512] = K-chunks x batch -- the matmul rhs layout directly.
    No PE transposes, no PSUM->SBUF copies.
  - rd = relu(X_dense @ dense_W.T + b) is folded in TRANSPOSED space: per
    K-chunk a tiny PE matmul preT = dwr_chunk.T @ xdt into PSUM, then one
    DVE op rhs = relu(preT) + gT_chunk. g_all is never mutated, so fm2
    never races the fold.
  - fm2 via the i<j triangle on DVE straight out of g_all (one
    scalar_tensor_tensor per (s, j>=1) with custom APs), fm1 via 4 strided
    reduces of col 416.
  - h1.T accumulated on PE over 104 K=128 chunks (tail chunks K=32), W1
    pre-permuted/chunk-packed on host and streamed in 8 SBUF slabs.
  - BatchNorm batch stats all-reduced across the 8 cores (two tiny
    AllReduces) with a dummy warm-up AllReduce at t=0 to absorb the first
    collective's setup latency under the gather phase.
"""

import os
import sys

for _p in ("/opt/trn_rl_repo",):
    if _p not in sys.path and os.path.isdir(_p):
        sys.path.insert(0, _p)

import numpy as np
import ml_dtypes

from concourse import bass, mybir
import concourse.tile as tile
from concourse import library_config, library_overlay
from concourse.vector_clock import ScopedClock
from concourse.bass_utils import run_bass_kernel_spmd

BF16 = mybir.dt.bfloat16
F32 = mybir.dt.float32
I16 = mybir.dt.int16
AF = mybir.ActivationFunctionType
OP = mybir.AluOpType

N_CORES = int(os.environ.get("DFM_N_CORES", "8"))
F = 26
V_FIELD = 8000
V = F * V_FIELD            # 208000
D = 16
FD = F * D                 # 416
ELEM = 512                 # G2 row: 416 features + fm1 col (416) + pad
FM1_COL = 416
DNN_IN = F * F * D         # 10816
H1, H2 = 256, 128
BS = 4096
SHARD = BS // 8            # 512
NS = SHARD // 128          # 4
NDENSE = 13
EPS = 1e-5

NGRP = 13                  # field groups of 2 (HBM gather = 1024 idxs)
NCHUNK = 4 * F             # 104 K-chunks of the main matmul
SLAB = 13                  # w1 chunks per SBUF slab
NSLAB = NCHUNK // SLAB     # 8


def _chunk_k(c):
    return 128 if c < 3 else 32


def _install_drain_split():
    """This container's walrus rejects >1 sync-wait per TPB_CTRL instruction;
    split the Tile kernel-tail drain's waits onto single-wait NOPs."""
    if getattr(tile.TileContext, "_dfm_drain_patched", False):
        return

    def _split_drain_and_barrier(self, tick_clock, wait_clock):
        collector = self.nc.sync.nop(nofuse=True)
        wait_clock.add_sem_waits(
            collector.ins, ScopedClock({None: tick_clock.global_clock})
        )
        si = collector.ins.sync_info
        waits = list(si.on_wait) if si is not None else []
        if len(waits) > 1:
            si.on_wait = waits[:1]
            for i in range(1, len(waits)):
                extra = self.nc.sync.nop(nofuse=True)
                extra.ins.sync_info = mybir.SyncInfo(
                    on_wait=[waits[i]], on_update=[]
                )
        self.nc.sync.drain()
        self.nc.all_engine_barrier()
        assert self.sems is not None
        popped = self.nc._tile_sem_poison_stack.pop()
        assert popped is self._sem_poison
        self.nc.clear_and_free_semaphores(list(self.sems.allocated().values()))
        self.nc.all_engine_barrier()

    tile.TileContext._drain_and_barrier = _split_drain_and_barrier
    tile.TileContext._dfm_drain_patched = True


def _split_multiwaits(nc, max_waits=1):
    """This walrus build also rejects >1 sync-wait on regular engine
    instructions: hoist extra waits onto single-wait NOPs just before."""
    n_split = 0
    for fn in nc.m.functions:
        for bb in fn.blocks:
            new_insts = []
            for inst in bb.instructions:
                si = getattr(inst, "sync_info", None)
                waits = list(si.on_wait) if si is not None and si.on_wait else []
                if len(waits) > max_waits:
                    keep = waits[-max_waits:]
                    for k, w in enumerate(waits[:-max_waits]):
                        nop = mybir.InstNoOp(
                            name=f"{inst.name}_w{k}",
                            engine=inst.engine,
                            sync_info=mybir.SyncInfo(on_wait=[w], on_update=[]),
                            bass_nofuse=True,
                        )
                        new_insts.append(nop)
                    si.on_wait = keep
                    n_split += 1
                new_insts.append(inst)
            bb.instructions[:] = new_insts
    return n_split


def build_program():
    _install_drain_split()
    nc = bass.Bass(num_swdge_queues=2)

    g2_d = nc.declare_dram_parameter("g2", [V, ELEM], BF16, isOutput=False)
    idx_d = nc.declare_dram_parameter("idx", [128, NGRP * 64], I16, isOutput=False)
    w1_d = nc.declare_dram_parameter("w1", [128, NCHUNK * H1], BF16, isOutput=False)
    dwr_d = nc.declare_dram_parameter("dwr", [NDENSE + 1, DNN_IN], BF16, isOutput=False)
    xdt_d = nc.declare_dram_parameter("xdt", [NDENSE + 1, SHARD], BF16, isOutput=False)
    w2_d = nc.declare_dram_parameter("w2", [128, H1], BF16, isOutput=False)
    wout_d = nc.declare_dram_parameter("wout", [128, 1], BF16, isOutput=False)
    fm1w_d = nc.declare_dram_parameter("fm1w", [NDENSE, 1], BF16, isOutput=False)
    bn1g_d = nc.declare_dram_parameter("bn1g", [128, 2], F32, isOutput=False)
    bn1b_d = nc.declare_dram_parameter("bn1b", [128, 2], F32, isOutput=False)
    bn2g_d = nc.declare_dram_parameter("bn2g", [128, 1], F32, isOutput=False)
    bn2b_d = nc.declare_dram_parameter("bn2b", [128, 1], F32, isOutput=False)
    c0_d = nc.declare_dram_parameter("c0", [128, 1], F32, isOutput=False)
    out_d = nc.declare_dram_parameter("out", [SHARD, 1], F32, isOutput=True)

    with tile.TileContext(nc) as tc:
        with (
            tc.tile_pool(name="persist", bufs=1) as persist,
            tc.tile_pool(name="w1p", bufs=3) as w1p,
            tc.tile_pool(name="gtp", bufs=3) as gtp,
            tc.tile_pool(name="rhsp", bufs=4) as rhsp,
            tc.tile_pool(name="scrp", bufs=2) as scrp,
            tc.tile_pool(name="small", bufs=2) as small,
            tc.tile_pool(name="ps_h1", bufs=1, space="PSUM") as ps_h1,
            tc.tile_pool(name="ps_pre", bufs=2, space="PSUM") as ps_pre,
            tc.tile_pool(name="ps_small", bufs=1, space="PSUM") as ps_small,
            tc.tile_pool(name="dram", bufs=1, space="DRAM") as dram,
        ):
            nc.gpsimd.load_library(library_config.mlp)

            # ---- persistent loads ----
            idx = persist.tile([128, NGRP * 64], I16, tag="idx")
            nc.sync.dma_start(idx[:], idx_d[:])
            dwr = persist.tile([NDENSE + 1, DNN_IN], BF16, tag="dwr")
            nc.sync.dma_start(dwr[:], dwr_d[:])
            xdt = persist.tile([NDENSE + 1, SHARD], BF16, tag="xdt")
            nc.sync.dma_start(xdt[:], xdt_d[:])
            w2 = persist.tile([128, H1], BF16, tag="w2")
            nc.sync.dma_start(w2[:], w2_d[:])
            wout = persist.tile([128, 1], BF16, tag="wout")
            nc.sync.dma_start(wout[:], wout_d[:])
            fm1w = persist.tile([NDENSE, 1], BF16, tag="fm1w")
            nc.sync.dma_start(fm1w[:], fm1w_d[:])
            bn1g = persist.tile([128, 2], F32, tag="bn1g")
            nc.sync.dma_start(bn1g[:], bn1g_d[:])
            bn1b = persist.tile([128, 2], F32, tag="bn1b")
            nc.sync.dma_start(bn1b[:], bn1b_d[:])
            bn2g = persist.tile([128, 1], F32, tag="bn2g")
            nc.sync.dma_start(bn2g[:], bn2g_d[:])
            bn2b = persist.tile([128, 1], F32, tag="bn2b")
            nc.sync.dma_start(bn2b[:], bn2b_d[:])
            c0 = persist.tile([128, 1], F32, tag="c0")
            nc.sync.dma_start(c0[:], c0_d[:])

            # ---- collective warm-up (hides first-CC setup under gathers) ----
            warm_in = dram.tile([128, 1], F32, tag="warm_i")
            warm_out = dram.tile([128, 1], F32, tag="warm_o")
            nc.sync.dma_start(warm_in[:], c0[:])
            nc.gpsimd.collective_compute(
                "AllReduce", OP.add,
                replica_groups=[list(range(N_CORES))],
                ins=[warm_in.opt()], outs=[warm_out.opt()],
            )

            r1024 = nc.gpsimd.to_reg(1024)

            # ---- gathers + main matmul, pipelined per 2-field group ----
            g_all = persist.tile([128, 8 * NGRP, ELEM], BF16, tag="g_all")
            gap = g_all[:].rearrange("p a b -> p (a b)")
            part0 = list(g_all[:].ap[0])
            gbase = g_all[:].offset

            def hbm_gather(k):
                nc.gpsimd.dma_gather(
                    g_all[:, 8 * k:8 * (k + 1), :],
                    g2_d[16000 * k:16000 * (k + 1), :],
                    idx[:, 64 * k:64 * (k + 1)],
                    1024, r1024, ELEM,
                )

            h1_ps = [
                ps_h1.tile([128, SHARD], F32, tag=f"h1_{h}", name=f"h1_ps{h}")
                for h in range(2)
            ]
            fm1e = persist.tile([128, NS], F32, tag="fm1e")
            Spart = persist.tile([128, NS, F], F32, tag="Spart")

            hbm_gather(0)
            hbm_gather(1)
            slabs = {}
            for sl in range(2):
                slabs[sl] = w1p.tile([128, SLAB * H1], BF16, tag="w1s",
                                     name=f"w1s{sl}")
                nc.sync.dma_start(
                    slabs[sl][:], w1_d[:, sl * SLAB * H1:(sl + 1) * SLAB * H1])

            q = 0
            for k in range(NGRP):
                if k + 2 < NGRP:
                    hbm_gather(k + 2)
                # fm2 for this group's two fields (reads g_all groups <= k)
                for jl in range(2):
                    j = 2 * k + jl
                    if j == 0:
                        continue
                    for s in range(NS):
                        in0 = bass.AP(
                            g_all[:].tensor, gbase + (4 * j + s) * ELEM,
                            [part0, [D, j], [1, D]],
                        )
                        in1 = bass.AP(
                            g_all[:].tensor, gbase + s * ELEM + D * j,
                            [part0, [4 * ELEM, j], [1, D]],
                        )
                        scr = scrp.tile([128, FD], BF16, tag="scr")
                        nc.vector.scalar_tensor_tensor(
                            out=scr[:, :j * D].rearrange("p (i d) -> p i d", d=D),
                            in0=in0, scalar=1.0, in1=in1,
                            op0=OP.mult, op1=OP.mult,
                            accum_out=Spart[:, s, j:j + 1],
                        )
                gt = gtp.tile([128, 32, 128], BF16, tag="gt")
                nc.sync.dma_start_transpose(gt[:], g_all[:, 8 * k:8 * (k + 1), :])
                gtv = gt[:].rearrange("p (jl s c) b -> p jl s c b", jl=2, s=NS)
                for jl in range(2):
                    j = 2 * k + jl
                    for c in range(4):
                        kc = _chunk_k(c)
                        sl, col = q // SLAB, (q % SLAB) * H1
                        if col == 0 and sl >= 2:
                            slabs[sl] = w1p.tile([128, SLAB * H1], BF16,
                                                 tag="w1s", name=f"w1s{sl}")
                            nc.sync.dma_start(
                                slabs[sl][:],
                                w1_d[:, sl * SLAB * H1:(sl + 1) * SLAB * H1])
                        pre = ps_pre.tile([128, SHARD], F32, tag="pre")
                        nc.tensor.matmul(
                            pre[0:kc, :],
                            lhsT=dwr[:, j * FD + c * 128: j * FD + c * 128 + kc],
                            rhs=xdt[:, :],
                            start=True, stop=True,
                        )
                        rhs = rhsp.tile([128, SHARD], BF16, tag="rhs")
                        nc.vector.scalar_tensor_tensor(
                            out=rhs[0:kc, :], in0=pre[0:kc, :], scalar=0.0,
                            in1=gtv[0:kc, jl, :, c, :], op0=OP.max, op1=OP.add,
                        )
                        for h in range(2):
                            nc.tensor.matmul(
                                h1_ps[h][:],
                                lhsT=slabs[sl][0:kc, col + h * 128: col + (h + 1) * 128],
                                rhs=rhs[0:kc, :],
                                start=(q == 0), stop=(q == NCHUNK - 1),
                            )
                        q += 1

            # ---- fm1 + fm2 reduction ----
            S_acc = small.tile([128, NS], F32, tag="S_acc")
            for s in range(NS):
                fm1_ap = bass.AP(
                    g_all[:].tensor, gbase + s * ELEM + FM1_COL,
                    [part0, [4 * ELEM, F]],
                )
                nc.vector.tensor_reduce(
                    out=fm1e[:, s:s + 1], in_=fm1_ap,
                    axis=mybir.AxisListType.X, op=OP.add,
                )
                nc.vector.tensor_reduce(
                    out=S_acc[:, s:s + 1], in_=Spart[:, s, 1:F],
                    axis=mybir.AxisListType.X, op=OP.add,
                )

            # ---- BN1 stats + allreduce ----
            stats1 = small.tile([128, 4], F32, tag="stats1")
            sq_scr = persist.tile([128, SHARD], F32, tag="sq")
            for h in range(2):
                nc.vector.tensor_reduce(
                    out=stats1[:, h:h + 1], in_=h1_ps[h][:],
                    axis=mybir.AxisListType.X, op=OP.add,
                )
                nc.scalar.activation(
                    out=sq_scr[:], in_=h1_ps[h][:], func=AF.Square,
                    accum_out=stats1[:, 2 + h:3 + h],
                )
            b1_in = dram.tile([128, 4], F32, tag="b1i")
            b1_out = dram.tile([128, 4], F32, tag="b1o")
            nc.sync.dma_start(b1_in[:], stats1[:])
            nc.gpsimd.collective_compute(
                "AllReduce", OP.add,
                replica_groups=[list(range(N_CORES))],
                ins=[b1_in.opt()], outs=[b1_out.opt()],
            )
            stats1g = small.tile([128, 4], F32, tag="stats1g")
            nc.sync.dma_start(stats1g[:], b1_out[:])

            def bn_scale_bias(statsg, col_s, col_q, gamma, beta, ncols):
                mean = small.tile([128, ncols], F32, tag="bn_mean")
                var = small.tile([128, ncols], F32, tag="bn_var")
                scale = small.tile([128, ncols], F32, tag="bn_scale")
                bias = small.tile([128, ncols], F32, tag="bn_bias")
                tmp = small.tile([128, ncols], F32, tag="bn_tmp")
                nc.vector.tensor_scalar_mul(
                    mean[:], statsg[:, col_s:col_s + ncols], 1.0 / BS)
                nc.vector.tensor_scalar_mul(
                    var[:], statsg[:, col_q:col_q + ncols], 1.0 / BS)
                nc.vector.tensor_tensor(
                    out=tmp[:], in0=mean[:], in1=mean[:], op=OP.mult)
                nc.vector.tensor_tensor(
                    out=var[:], in0=var[:], in1=tmp[:], op=OP.subtract)
                nc.vector.tensor_scalar_add(var[:], var[:], EPS)
                nc.vector.reciprocal(tmp[:], var[:])
                nc.scalar.activation(out=tmp[:], in_=tmp[:], func=AF.Sqrt)
                nc.vector.tensor_tensor(
                    out=scale[:], in0=gamma[:], in1=tmp[:], op=OP.mult)
                nc.vector.tensor_tensor(
                    out=tmp[:], in0=mean[:], in1=scale[:], op=OP.mult)
                nc.vector.tensor_tensor(
                    out=bias[:], in0=beta[:], in1=tmp[:], op=OP.subtract)
                return scale, bias

            sc1, bi1 = bn_scale_bias(stats1g, 0, 2, bn1g, bn1b, 2)
            h1r = persist.tile([128, 2, SHARD], BF16, tag="h1r")
            for h in range(2):
                nc.scalar.activation(
                    out=h1r[:, h, :], in_=h1_ps[h][:], func=AF.Relu,
                    bias=bi1[:, h:h + 1], scale=sc1[:, h:h + 1],
                )

            # ---- layer 2 ----
            h2_ps = ps_small.tile([128, SHARD], F32, tag="h2")
            for h in range(2):
                nc.tensor.matmul(
                    h2_ps[:],
                    lhsT=w2[:, h * 128:(h + 1) * 128],
                    rhs=h1r[:, h, :],
                    start=(h == 0), stop=(h == 1),
                )
            stats2 = small.tile([128, 2], F32, tag="stats2")
            nc.vector.tensor_reduce(
                out=stats2[:, 0:1], in_=h2_ps[:],
                axis=mybir.AxisListType.X, op=OP.add,
            )
            nc.scalar.activation(
                out=sq_scr[:], in_=h2_ps[:], func=AF.Square,
                accum_out=stats2[:, 1:2],
            )
            b2_in = dram.tile([128, 2], F32, tag="b2i")
            b2_out = dram.tile([128, 2], F32, tag="b2o")
            nc.sync.dma_start(b2_in[:], stats2[:])
            nc.gpsimd.collective_compute(
                "AllReduce", OP.add,
                replica_groups=[list(range(N_CORES))],
                ins=[b2_in.opt()], outs=[b2_out.opt()],
            )
            stats2g = small.tile([128, 2], F32, tag="stats2g")
            nc.sync.dma_start(stats2g[:], b2_out[:])
            sc2, bi2 = bn_scale_bias(stats2g, 0, 1, bn2g, bn2b, 1)
            h2r = persist.tile([128, SHARD], BF16, tag="h2r")
            nc.scalar.activation(
                out=h2r[:], in_=h2_ps[:], func=AF.Relu,
                bias=bi2[:, 0:1], scale=sc2[:, 0:1],
            )

            # ---- heads ----
            head_ps = ps_small.tile([128, 2 * NS], F32, tag="heads")
            for s in range(NS):
                nc.tensor.matmul(
                    head_ps[:, s:s + 1],
                    lhsT=h2r[:, s * 128:(s + 1) * 128],
                    rhs=wout[:],
                    start=True, stop=True,
                )
                nc.tensor.matmul(
                    head_ps[:, NS + s:NS + s + 1],
                    lhsT=xdt[0:NDENSE, s * 128:(s + 1) * 128],
                    rhs=fm1w[:],
                    start=True, stop=True,
                )

            tot = small.tile([128, NS], F32, tag="tot")
            res = small.tile([128, NS], F32, tag="res")
            nc.vector.tensor_tensor(
                out=tot[:], in0=fm1e[:], in1=head_ps[:, 0:NS], op=OP.add)
            nc.vector.tensor_tensor(
                out=tot[:], in0=tot[:], in1=head_ps[:, NS:2 * NS], op=OP.add)
            nc.vector.tensor_tensor(
                out=tot[:], in0=tot[:], in1=S_acc[:], op=OP.add)
            nc.scalar.activation(
                out=res[:], in_=tot[:], func=AF.Sigmoid,
                bias=c0[:, 0:1], scale=1.0,
            )
            out_ap = out_d[:, :].rearrange("(s p) o -> p (s o)", p=128)
            nc.sync.dma_start(out_ap, res[:])

    library_overlay.lower_extended_insts(nc)
    _split_multiwaits(nc)
    return nc


_NC_CACHE = None


def _get_nc():
    global _NC_CACHE
    if _NC_CACHE is None:
        _NC_CACHE = build_program()
    return _NC_CACHE


def _wrap_idx16(vals):
    """[n] -> [128, n//16] int16: idx i at [i%16, i//16], replicated x8."""
    n = len(vals)
    a = np.asarray(vals, dtype=np.int16).reshape(n // 16, 16).T
    return np.tile(a, (8, 1))


def make_in_maps(X_sparse, X_dense, fm1_emb, bias, fm1_dense_W, fm1_dense_b,
                 emb_tables, dense_W, dense_b,
                 W1, b1, g1, beta1, W2, b2, g2, beta2, Wout, bout):
    bf16 = ml_dtypes.bfloat16
    f32 = np.float32

    g2t = np.zeros((V, ELEM), dtype=bf16)
    g2t[:, 0:FD] = (
        np.ascontiguousarray(emb_tables.transpose(1, 0, 2)).reshape(V, FD)
        .astype(bf16)
    )
    g2t[:, FM1_COL] = fm1_emb[:, 0].astype(bf16)

    # W1 permuted to (j, f=16i+d) rows, packed into 104 K-chunks of 128.
    W1p = np.ascontiguousarray(
        W1.reshape(H1, F, F, D).transpose(2, 1, 3, 0)
    ).reshape(F, FD, H1)
    w1k = np.zeros((NCHUNK, 128, H1), dtype=f32)
    for j in range(F):
        for c in range(4):
            kc = _chunk_k(c)
            w1k[4 * j + c, 0:kc] = W1p[j, c * 128:c * 128 + kc]
    w1h = np.ascontiguousarray(w1k.transpose(1, 0, 2)).reshape(
        128, NCHUNK * H1).astype(bf16)

    dWr = np.ascontiguousarray(
        dense_W.reshape(F, F, D, NDENSE).transpose(1, 0, 2, 3)
    ).reshape(DNN_IN, NDENSE)
    dwrh = np.zeros((NDENSE + 1, DNN_IN), dtype=bf16)
    dwrh[0:NDENSE] = dWr.T.astype(bf16)
    dwrh[NDENSE] = np.ascontiguousarray(
        dense_b.reshape(F, F, D).transpose(1, 0, 2)
    ).reshape(DNN_IN).astype(bf16)

    w2h = np.ascontiguousarray(
        W2.T.reshape(2, 128, H2).transpose(1, 0, 2)
    ).reshape(128, H1).astype(bf16)
    wouth = Wout.reshape(H2, 1).astype(bf16) if Wout.shape == (H2, 1) else \
        Wout.T.astype(bf16)
    fm1wh = fm1_dense_W.T.astype(bf16)  # [13, 1]

    bn1gh = np.ascontiguousarray(g1.reshape(2, 128).T).astype(f32)
    bn1bh = np.ascontiguousarray(beta1.reshape(2, 128).T).astype(f32)
    bn2gh = g2.reshape(128, 1).astype(f32)
    bn2bh = beta2.reshape(128, 1).astype(f32)
    c0h = np.full((128, 1),
                  float(bias[0]) + float(fm1_dense_b[0]) + float(bout[0]),
                  dtype=f32)

    in_maps = []
    for core in range(N_CORES):
        Xl = X_sparse[core * SHARD:(core + 1) * SHARD]   # [512, 26] local
        idx_h = np.zeros((128, NGRP * 64), dtype=np.int16)
        for k in range(NGRP):
            vals = np.empty(1024, dtype=np.int64)
            for jl in range(2):
                vals[jl * 512:(jl + 1) * 512] = Xl[:, 2 * k + jl] + jl * V_FIELD
            idx_h[:, 64 * k:64 * (k + 1)] = _wrap_idx16(vals)
        xdt_c = np.ones((NDENSE + 1, SHARD), dtype=bf16)
        xdt_c[0:NDENSE] = X_dense[core * SHARD:(core + 1) * SHARD].T.astype(bf16)
        in_maps.append({
            "g2": g2t, "idx": idx_h, "w1": w1h, "dwr": dwrh,
            "xdt": xdt_c, "w2": w2h, "wout": wouth, "fm1w": fm1wh,
            "bn1g": bn1gh, "bn1b": bn1bh, "bn2g": bn2gh, "bn2b": bn2bh,
            "c0": c0h,
        })
    return in_maps


def kernel(**inputs):
    nc = _get_nc()
    in_maps = make_in_maps(**{k: np.asarray(v) for k, v in inputs.items()})
    res = run_bass_kernel_spmd(
        nc, in_maps, core_ids=list(range(N_CORES)),
        trace=bool(int(os.environ.get("DFM_TRACE", "0"))),
    )
    out = np.concatenate([res.results[c]["out"] for c in range(N_CORES)], axis=0)
    kernel.last_results = res
    return out.astype(np.float32)


# revision 6
# speedup vs baseline: 1.3030x; 1.3030x over previous
"""Trainium2 Bass kernel for nn_DeepFM_3066606649824.

Strategy (8 NeuronCores, data-parallel over batch; SHARD = 512 rows/core):
  - Host: restructure the 26 FFM tables [26, 208000, 16] f32 into one bf16
    row-major table G2 [208000, 512]: cols 0:416 = the 26 tables' rows for
    that index (feature f = 16*i + d), col 416 = fm1_emb, rest zero.
    1024B rows satisfy dma_gather's 256B-multiple elem/stride rule.
  - Gather via InstDMAGatherAnt (Q7 'mlp' library): 13 HBM gathers of 1024
    rows (2 fields x 512 batch) land g_all [128, 104rows, 512] (row 4j+s),
    then 26 SBUF-source TRANSPOSED dma_gathers (512 idxs each, <=512 per
    instr is a Q7 limit; HBM gathers cap at 1024) produce gT tiles
    [128, 4, 512] = K-chunks x batch -- the matmul rhs layout directly.
    No PE transposes, no PSUM->SBUF copies.
  - rd = relu(X_dense @ dense_W.T + b) is folded in TRANSPOSED space: per
    K-chunk a tiny PE matmul preT = dwr_chunk.T @ xdt into PSUM, then one
    DVE op rhs = relu(preT) + gT_chunk. g_all is never mutated, so fm2
    never races the fold.
  - fm2 via the i<j triangle on DVE straight out of g_all (one
    scalar_tensor_tensor per (s, j>=1) with custom APs), fm1 via 4 strided
    reduces of col 416.
  - h1.T accumulated on PE over 104 K=128 chunks (tail chunks K=32), W1
    pre-permuted/chunk-packed on host and streamed in 8 SBUF slabs.
  - BatchNorm batch stats all-reduced across the 8 cores (two tiny
    AllReduces) with a dummy warm-up AllReduce at t=0 to absorb the first
    collective's setup latency under the gather phase.
"""

import os
import sys

for _p in ("/opt/trn_rl_repo",):
    if _p not in sys.path and os.path.isdir(_p):
        sys.path.insert(0, _p)

import numpy as np
import ml_dtypes

from concourse import bass, mybir
import concourse.tile as tile
from concourse import library_config, library_overlay
from concourse.vector_clock import ScopedClock
from concourse.bass_utils import run_bass_kernel_spmd

BF16 = mybir.dt.bfloat16
F32 = mybir.dt.float32
I16 = mybir.dt.int16
AF = mybir.ActivationFunctionType
OP = mybir.AluOpType

N_CORES = int(os.environ.get("DFM_N_CORES", "8"))
F = 26
V_FIELD = 8000
V = F * V_FIELD            # 208000
D = 16
FD = F * D                 # 416
ELEM = 512                 # G2 row: 416 features + fm1 col (416) + pad
FM1_COL = 416
DNN_IN = F * F * D         # 10816
H1, H2 = 256, 128
BS = 4096
SHARD = BS // 8            # 512
NS = SHARD // 128          # 4
NDENSE = 13
EPS = 1e-5

NGRP = 13                  # field groups of 2 (HBM gather = 1024 idxs)
NCHUNK = 4 * F             # 104 K-chunks of the main matmul
SLAB = 13                  # w1 chunks per SBUF slab
NSLAB = NCHUNK // SLAB     # 8


def _chunk_k(c):
    return 128 if c < 3 else 32


def _install_drain_split():
    """This container's walrus rejects >1 sync-wait per TPB_CTRL instruction;
    split the Tile kernel-tail drain's waits onto single-wait NOPs."""
    if getattr(tile.TileContext, "_dfm_drain_patched", False):
        return

    def _split_drain_and_barrier(self, tick_clock, wait_clock):
        collector = self.nc.sync.nop(nofuse=True)
        wait_clock.add_sem_waits(
            collector.ins, ScopedClock({None: tick_clock.global_clock})
        )
        si = collector.ins.sync_info
        waits = list(si.on_wait) if si is not None else []
        if len(waits) > 1:
            si.on_wait = waits[:1]
            for i in range(1, len(waits)):
                extra = self.nc.sync.nop(nofuse=True)
                extra.ins.sync_info = mybir.SyncInfo(
                    on_wait=[waits[i]], on_update=[]
                )
        self.nc.sync.drain()
        self.nc.all_engine_barrier()
        assert self.sems is not None
        popped = self.nc._tile_sem_poison_stack.pop()
        assert popped is self._sem_poison
        self.nc.clear_and_free_semaphores(list(self.sems.allocated().values()))
        self.nc.all_engine_barrier()

    tile.TileContext._drain_and_barrier = _split_drain_and_barrier
    tile.TileContext._dfm_drain_patched = True


def _split_multiwaits(nc, max_waits=1):
    """This walrus build also rejects >1 sync-wait on regular engine
    instructions: hoist extra waits onto single-wait NOPs just before."""
    n_split = 0
    for fn in nc.m.functions:
        for bb in fn.blocks:
            new_insts = []
            for inst in bb.instructions:
                si = getattr(inst, "sync_info", None)
                waits = list(si.on_wait) if si is not None and si.on_wait else []
                if len(waits) > max_waits:
                    keep = waits[-max_waits:]
                    for k, w in enumerate(waits[:-max_waits]):
                        nop = mybir.InstNoOp(
                            name=f"{inst.name}_w{k}",
                            engine=inst.engine,
                            sync_info=mybir.SyncInfo(on_wait=[w], on_update=[]),
                            bass_nofuse=True,
                        )
                        new_insts.append(nop)
                    si.on_wait = keep
                    n_split += 1
                new_insts.append(inst)
            bb.instructions[:] = new_insts
    return n_split


def build_program():
    _install_drain_split()
    nc = bass.Bass(num_swdge_queues=2)

    g2_d = nc.declare_dram_parameter("g2", [V, ELEM], BF16, isOutput=False)
    idx_d = nc.declare_dram_parameter("idx", [128, NGRP * 64], I16, isOutput=False)
    idxt_d = nc.declare_dram_parameter("idxt", [128, 16], I16, isOutput=False)
    w1_d = nc.declare_dram_parameter("w1", [128, NCHUNK * H1], BF16, isOutput=False)
    dwr_d = nc.declare_dram_parameter("dwr", [NDENSE + 1, DNN_IN], BF16, isOutput=False)
    xdt_d = nc.declare_dram_parameter("xdt", [NDENSE + 1, SHARD], BF16, isOutput=False)
    w2_d = nc.declare_dram_parameter("w2", [128, H1], BF16, isOutput=False)
    wout_d = nc.declare_dram_parameter("wout", [128, 1], BF16, isOutput=False)
    fm1w_d = nc.declare_dram_parameter("fm1w", [NDENSE, 1], BF16, isOutput=False)
    bn1g_d = nc.declare_dram_parameter("bn1g", [128, 2], F32, isOutput=False)
    bn1b_d = nc.declare_dram_parameter("bn1b", [128, 2], F32, isOutput=False)
    bn2g_d = nc.declare_dram_parameter("bn2g", [128, 1], F32, isOutput=False)
    bn2b_d = nc.declare_dram_parameter("bn2b", [128, 1], F32, isOutput=False)
    c0_d = nc.declare_dram_parameter("c0", [128, 1], F32, isOutput=False)
    out_d = nc.declare_dram_parameter("out", [SHARD, 1], F32, isOutput=True)

    with tile.TileContext(nc) as tc:
        with (
            tc.tile_pool(name="persist", bufs=1) as persist,
            tc.tile_pool(name="w1p", bufs=3) as w1p,
            tc.tile_pool(name="gtp", bufs=3) as gtp,
            tc.tile_pool(name="rhsp", bufs=4) as rhsp,
            tc.tile_pool(name="scrp", bufs=2) as scrp,
            tc.tile_pool(name="small", bufs=2) as small,
            tc.tile_pool(name="ps_h1", bufs=1, space="PSUM") as ps_h1,
            tc.tile_pool(name="ps_pre", bufs=2, space="PSUM") as ps_pre,
            tc.tile_pool(name="ps_small", bufs=1, space="PSUM") as ps_small,
            tc.tile_pool(name="dram", bufs=1, space="DRAM") as dram,
        ):
            nc.gpsimd.load_library(library_config.mlp)

            # ---- persistent loads ----
            idx = persist.tile([128, NGRP * 64], I16, tag="idx")
            nc.sync.dma_start(idx[:], idx_d[:])
            idxt = persist.tile([128, 16], I16, tag="idxt")
            nc.sync.dma_start(idxt[:], idxt_d[:])
            dwr = persist.tile([NDENSE + 1, DNN_IN], BF16, tag="dwr")
            nc.sync.dma_start(dwr[:], dwr_d[:])
            xdt = persist.tile([NDENSE + 1, SHARD], BF16, tag="xdt")
            nc.sync.dma_start(xdt[:], xdt_d[:])
            w2 = persist.tile([128, H1], BF16, tag="w2")
            nc.sync.dma_start(w2[:], w2_d[:])
            wout = persist.tile([128, 1], BF16, tag="wout")
            nc.sync.dma_start(wout[:], wout_d[:])
            fm1w = persist.tile([NDENSE, 1], BF16, tag="fm1w")
            nc.sync.dma_start(fm1w[:], fm1w_d[:])
            bn1g = persist.tile([128, 2], F32, tag="bn1g")
            nc.sync.dma_start(bn1g[:], bn1g_d[:])
            bn1b = persist.tile([128, 2], F32, tag="bn1b")
            nc.sync.dma_start(bn1b[:], bn1b_d[:])
            bn2g = persist.tile([128, 1], F32, tag="bn2g")
            nc.sync.dma_start(bn2g[:], bn2g_d[:])
            bn2b = persist.tile([128, 1], F32, tag="bn2b")
            nc.sync.dma_start(bn2b[:], bn2b_d[:])
            c0 = persist.tile([128, 1], F32, tag="c0")
            nc.sync.dma_start(c0[:], c0_d[:])

            # ---- collective warm-up (hides first-CC setup under gathers) ----
            warm_in = dram.tile([128, 1], F32, tag="warm_i")
            warm_out = dram.tile([128, 1], F32, tag="warm_o")
            nc.sync.dma_start(warm_in[:], c0[:])
            nc.gpsimd.collective_compute(
                "AllReduce", OP.add,
                replica_groups=[list(range(N_CORES))],
                ins=[warm_in.opt()], outs=[warm_out.opt()],
            )

            r1024 = nc.gpsimd.to_reg(1024)
            r128 = nc.gpsimd.to_reg(128)

            # ---- gathers + main matmul, pipelined per 2-field group ----
            g_all = persist.tile([128, 8 * NGRP, ELEM], BF16, tag="g_all")
            gap = g_all[:].rearrange("p a b -> p (a b)")
            part0 = list(g_all[:].ap[0])
            gbase = g_all[:].offset

            def hbm_gather(k):
                nc.gpsimd.dma_gather(
                    g_all[:, 8 * k:8 * (k + 1), :],
                    g2_d[16000 * k:16000 * (k + 1), :],
                    idx[:, 64 * k:64 * (k + 1)],
                    1024, r1024, ELEM,
                )

            h1_ps = [
                ps_h1.tile([128, SHARD], F32, tag=f"h1_{h}", name=f"h1_ps{h}")
                for h in range(2)
            ]
            fm1e = persist.tile([128, NS], F32, tag="fm1e")
            Spart = persist.tile([128, NS, F], F32, tag="Spart")

            hbm_gather(0)
            hbm_gather(1)
            slabs = {}
            for sl in range(2):
                slabs[sl] = w1p.tile([128, SLAB * H1], BF16, tag="w1s",
                                     name=f"w1s{sl}")
                nc.sync.dma_start(
                    slabs[sl][:], w1_d[:, sl * SLAB * H1:(sl + 1) * SLAB * H1])

            q = 0
            for k in range(NGRP):
                if k + 2 < NGRP:
                    hbm_gather(k + 2)
                # fm2 for this group's two fields (reads g_all groups <= k)
                for jl in range(2):
                    j = 2 * k + jl
                    if j == 0:
                        continue
                    for s in range(NS):
                        in0 = bass.AP(
                            g_all[:].tensor, gbase + (4 * j + s) * ELEM,
                            [part0, [D, j], [1, D]],
                        )
                        in1 = bass.AP(
                            g_all[:].tensor, gbase + s * ELEM + D * j,
                            [part0, [4 * ELEM, j], [1, D]],
                        )
                        scr = scrp.tile([128, FD], BF16, tag="scr")
                        nc.vector.scalar_tensor_tensor(
                            out=scr[:, :j * D].rearrange("p (i d) -> p i d", d=D),
                            in0=in0, scalar=1.0, in1=in1,
                            op0=OP.mult, op1=OP.mult,
                            accum_out=Spart[:, s, j:j + 1],
                        )
                for jl in range(2):
                    j = 2 * k + jl
                    gt = gtp.tile([128, 16, 128], BF16, tag="gt")
                    nc.gpsimd.dma_gather(
                        gt[:],
                        g_all[:, 8 * k:8 * (k + 1), :].rearrange("p a b -> p (a b)"),
                        idxt[:, jl * 8:(jl + 1) * 8],
                        128, r128, 4 * ELEM,
                        transpose=True,
                        sbuf_tokens_per_rank=128,
                        sbuf_free_dim_per_rank=4 * ELEM * 2,
                        queue_num=1,
                    )
                    gtv = gt[:].rearrange("p (s c) b -> p s c b", s=NS)
                    for c in range(4):
                        kc = _chunk_k(c)
                        sl, col = q // SLAB, (q % SLAB) * H1
                        if col == 0 and sl >= 2:
                            slabs[sl] = w1p.tile([128, SLAB * H1], BF16,
                                                 tag="w1s", name=f"w1s{sl}")
                            nc.sync.dma_start(
                                slabs[sl][:],
                                w1_d[:, sl * SLAB * H1:(sl + 1) * SLAB * H1])
                        pre = ps_pre.tile([128, SHARD], F32, tag="pre")
                        nc.tensor.matmul(
                            pre[0:kc, :],
                            lhsT=dwr[:, j * FD + c * 128: j * FD + c * 128 + kc],
                            rhs=xdt[:, :],
                            start=True, stop=True,
                        )
                        rhs = rhsp.tile([128, SHARD], BF16, tag="rhs")
                        nc.vector.scalar_tensor_tensor(
                            out=rhs[0:kc, :], in0=pre[0:kc, :], scalar=0.0,
                            in1=gtv[0:kc, :, c, :], op0=OP.max, op1=OP.add,
                        )
                        for h in range(2):
                            nc.tensor.matmul(
                                h1_ps[h][:],
                                lhsT=slabs[sl][0:kc, col + h * 128: col + (h + 1) * 128],
                                rhs=rhs[0:kc, :],
                                start=(q == 0), stop=(q == NCHUNK - 1),
                            )
                        q += 1

            # ---- fm1 + fm2 reduction ----
            S_acc = small.tile([128, NS], F32, tag="S_acc")
            for s in range(NS):
                fm1_ap = bass.AP(
                    g_all[:].tensor, gbase + s * ELEM + FM1_COL,
                    [part0, [4 * ELEM, F]],
                )
                nc.vector.tensor_reduce(
                    out=fm1e[:, s:s + 1], in_=fm1_ap,
                    axis=mybir.AxisListType.X, op=OP.add,
                )
                nc.vector.tensor_reduce(
                    out=S_acc[:, s:s + 1], in_=Spart[:, s, 1:F],
                    axis=mybir.AxisListType.X, op=OP.add,
                )

            # ---- BN1 stats + allreduce ----
            stats1 = small.tile([128, 4], F32, tag="stats1")
            sq_scr = persist.tile([128, SHARD], F32, tag="sq")
            for h in range(2):
                nc.vector.tensor_reduce(
                    out=stats1[:, h:h + 1], in_=h1_ps[h][:],
                    axis=mybir.AxisListType.X, op=OP.add,
                )
                nc.scalar.activation(
                    out=sq_scr[:], in_=h1_ps[h][:], func=AF.Square,
                    accum_out=stats1[:, 2 + h:3 + h],
                )
            b1_in = dram.tile([128, 4], F32, tag="b1i")
            b1_out = dram.tile([128, 4], F32, tag="b1o")
            nc.sync.dma_start(b1_in[:], stats1[:])
            nc.gpsimd.collective_compute(
                "AllReduce", OP.add,
                replica_groups=[list(range(N_CORES))],
                ins=[b1_in.opt()], outs=[b1_out.opt()],
            )
            stats1g = small.tile([128, 4], F32, tag="stats1g")
            nc.sync.dma_start(stats1g[:], b1_out[:])

            def bn_scale_bias(statsg, col_s, col_q, gamma, beta, ncols):
                mean = small.tile([128, ncols], F32, tag="bn_mean")
                var = small.tile([128, ncols], F32, tag="bn_var")
                scale = small.tile([128, ncols], F32, tag="bn_scale")
                bias = small.tile([128, ncols], F32, tag="bn_bias")
                tmp = small.tile([128, ncols], F32, tag="bn_tmp")
                nc.vector.tensor_scalar_mul(
                    mean[:], statsg[:, col_s:col_s + ncols], 1.0 / BS)
                nc.vector.tensor_scalar_mul(
                    var[:], statsg[:, col_q:col_q + ncols], 1.0 / BS)
                nc.vector.tensor_tensor(
                    out=tmp[:], in0=mean[:], in1=mean[:], op=OP.mult)
                nc.vector.tensor_tensor(
                    out=var[:], in0=var[:], in1=tmp[:], op=OP.subtract)
                nc.vector.tensor_scalar_add(var[:], var[:], EPS)
                nc.vector.reciprocal(tmp[:], var[:])
                nc.scalar.activation(out=tmp[:], in_=tmp[:], func=AF.Sqrt)
                nc.vector.tensor_tensor(
                    out=scale[:], in0=gamma[:], in1=tmp[:], op=OP.mult)
                nc.vector.tensor_tensor(
                    out=tmp[:], in0=mean[:], in1=scale[:], op=OP.mult)
                nc.vector.tensor_tensor(
                    out=bias[:], in0=beta[:], in1=tmp[:], op=OP.subtract)
                return scale, bias

            sc1, bi1 = bn_scale_bias(stats1g, 0, 2, bn1g, bn1b, 2)
            h1r = persist.tile([128, 2, SHARD], BF16, tag="h1r")
            for h in range(2):
                nc.scalar.activation(
                    out=h1r[:, h, :], in_=h1_ps[h][:], func=AF.Relu,
                    bias=bi1[:, h:h + 1], scale=sc1[:, h:h + 1],
                )

            # ---- layer 2 ----
            h2_ps = ps_small.tile([128, SHARD], F32, tag="h2")
            for h in range(2):
                nc.tensor.matmul(
                    h2_ps[:],
                    lhsT=w2[:, h * 128:(h + 1) * 128],
                    rhs=h1r[:, h, :],
                    start=(h == 0), stop=(h == 1),
                )
            stats2 = small.tile([128, 2], F32, tag="stats2")
            nc.vector.tensor_reduce(
                out=stats2[:, 0:1], in_=h2_ps[:],
                axis=mybir.AxisListType.X, op=OP.add,
            )
            nc.scalar.activation(
                out=sq_scr[:], in_=h2_ps[:], func=AF.Square,
                accum_out=stats2[:, 1:2],
            )
            b2_in = dram.tile([128, 2], F32, tag="b2i")
            b2_out = dram.tile([128, 2], F32, tag="b2o")
            nc.sync.dma_start(b2_in[:], stats2[:])
            nc.gpsimd.collective_compute(
                "AllReduce", OP.add,
                replica_groups=[list(range(N_CORES))],
                ins=[b2_in.opt()], outs=[b2_out.opt()],
            )
            stats2g = small.tile([128, 2], F32, tag="stats2g")
            nc.sync.dma_start(stats2g[:], b2_out[:])
            sc2, bi2 = bn_scale_bias(stats2g, 0, 1, bn2g, bn2b, 1)
            h2r = persist.tile([128, SHARD], BF16, tag="h2r")
            nc.scalar.activation(
                out=h2r[:], in_=h2_ps[:], func=AF.Relu,
                bias=bi2[:, 0:1], scale=sc2[:, 0:1],
            )

            # ---- heads ----
            head_ps = ps_small.tile([128, 2 * NS], F32, tag="heads")
            for s in range(NS):
                nc.tensor.matmul(
                    head_ps[:, s:s + 1],
                    lhsT=h2r[:, s * 128:(s + 1) * 128],
                    rhs=wout[:],
                    start=True, stop=True,
                )
                nc.tensor.matmul(
                    head_ps[:, NS + s:NS + s + 1],
                    lhsT=xdt[0:NDENSE, s * 128:(s + 1) * 128],
                    rhs=fm1w[:],
                    start=True, stop=True,
                )

            tot = small.tile([128, NS], F32, tag="tot")
            res = small.tile([128, NS], F32, tag="res")
            nc.vector.tensor_tensor(
                out=tot[:], in0=fm1e[:], in1=head_ps[:, 0:NS], op=OP.add)
            nc.vector.tensor_tensor(
                out=tot[:], in0=tot[:], in1=head_ps[:, NS:2 * NS], op=OP.add)
            nc.vector.tensor_tensor(
                out=tot[:], in0=tot[:], in1=S_acc[:], op=OP.add)
            nc.scalar.activation(
                out=res[:], in_=tot[:], func=AF.Sigmoid,
                bias=c0[:, 0:1], scale=1.0,
            )
            out_ap = out_d[:, :].rearrange("(s p) o -> p (s o)", p=128)
            nc.sync.dma_start(out_ap, res[:])

    library_overlay.lower_extended_insts(nc)
    _split_multiwaits(nc)
    return nc


_NC_CACHE = None


def _get_nc():
    global _NC_CACHE
    if _NC_CACHE is None:
        _NC_CACHE = build_program()
    return _NC_CACHE


def _wrap_idx16(vals):
    """[n] -> [128, n//16] int16: idx i at [i%16, i//16], replicated x8."""
    n = len(vals)
    a = np.asarray(vals, dtype=np.int16).reshape(n // 16, 16).T
    return np.tile(a, (8, 1))


def make_in_maps(X_sparse, X_dense, fm1_emb, bias, fm1_dense_W, fm1_dense_b,
                 emb_tables, dense_W, dense_b,
                 W1, b1, g1, beta1, W2, b2, g2, beta2, Wout, bout):
    bf16 = ml_dtypes.bfloat16
    f32 = np.float32

    g2t = np.zeros((V, ELEM), dtype=bf16)
    g2t[:, 0:FD] = (
        np.ascontiguousarray(emb_tables.transpose(1, 0, 2)).reshape(V, FD)
        .astype(bf16)
    )
    g2t[:, FM1_COL] = fm1_emb[:, 0].astype(bf16)

    # W1 permuted to (j, f=16i+d) rows, packed into 104 K-chunks of 128.
    W1p = np.ascontiguousarray(
        W1.reshape(H1, F, F, D).transpose(2, 1, 3, 0)
    ).reshape(F, FD, H1)
    w1k = np.zeros((NCHUNK, 128, H1), dtype=f32)
    for j in range(F):
        for c in range(4):
            kc = _chunk_k(c)
            w1k[4 * j + c, 0:kc] = W1p[j, c * 128:c * 128 + kc]
    w1h = np.ascontiguousarray(w1k.transpose(1, 0, 2)).reshape(
        128, NCHUNK * H1).astype(bf16)

    dWr = np.ascontiguousarray(
        dense_W.reshape(F, F, D, NDENSE).transpose(1, 0, 2, 3)
    ).reshape(DNN_IN, NDENSE)
    dwrh = np.zeros((NDENSE + 1, DNN_IN), dtype=bf16)
    dwrh[0:NDENSE] = dWr.T.astype(bf16)
    dwrh[NDENSE] = np.ascontiguousarray(
        dense_b.reshape(F, F, D).transpose(1, 0, 2)
    ).reshape(DNN_IN).astype(bf16)

    w2h = np.ascontiguousarray(
        W2.T.reshape(2, 128, H2).transpose(1, 0, 2)
    ).reshape(128, H1).astype(bf16)
    wouth = Wout.reshape(H2, 1).astype(bf16) if Wout.shape == (H2, 1) else \
        Wout.T.astype(bf16)
    fm1wh = fm1_dense_W.T.astype(bf16)  # [13, 1]

    bn1gh = np.ascontiguousarray(g1.reshape(2, 128).T).astype(f32)
    bn1bh = np.ascontiguousarray(beta1.reshape(2, 128).T).astype(f32)
    bn2gh = g2.reshape(128, 1).astype(f32)
    bn2bh = beta2.reshape(128, 1).astype(f32)
    c0h = np.full((128, 1),
                  float(bias[0]) + float(fm1_dense_b[0]) + float(bout[0]),
                  dtype=f32)

    idxt_h = np.concatenate(
        [_wrap_idx16(np.arange(128) + 128 * jl) for jl in range(2)], axis=1)

    in_maps = []
    for core in range(N_CORES):
        Xl = X_sparse[core * SHARD:(core + 1) * SHARD]   # [512, 26] local
        idx_h = np.zeros((128, NGRP * 64), dtype=np.int16)
        for k in range(NGRP):
            vals = np.empty(1024, dtype=np.int64)
            for jl in range(2):
                vals[jl * 512:(jl + 1) * 512] = Xl[:, 2 * k + jl] + jl * V_FIELD
            idx_h[:, 64 * k:64 * (k + 1)] = _wrap_idx16(vals)
        xdt_c = np.ones((NDENSE + 1, SHARD), dtype=bf16)
        xdt_c[0:NDENSE] = X_dense[core * SHARD:(core + 1) * SHARD].T.astype(bf16)
        in_maps.append({
            "g2": g2t, "idx": idx_h, "idxt": idxt_h, "w1": w1h, "dwr": dwrh,
            "xdt": xdt_c, "w2": w2h, "wout": wouth, "fm1w": fm1wh,
            "bn1g": bn1gh, "bn1b": bn1bh, "bn2g": bn2gh, "bn2b": bn2bh,
            "c0": c0h,
        })
    return in_maps


def kernel(**inputs):
    nc = _get_nc()
    in_maps = make_in_maps(**{k: np.asarray(v) for k, v in inputs.items()})
    res = run_bass_kernel_spmd(
        nc, in_maps, core_ids=list(range(N_CORES)),
        trace=bool(int(os.environ.get("DFM_TRACE", "0"))),
    )
    out = np.concatenate([res.results[c]["out"] for c in range(N_CORES)], axis=0)
    kernel.last_results = res
    return out.astype(np.float32)


# revision 8
# speedup vs baseline: 1.4128x; 1.0843x over previous
"""Trainium2 Bass kernel for nn_DeepFM_3066606649824.

Strategy (8 NeuronCores, data-parallel over batch; SHARD = 512 rows/core):
  - Host: restructure the 26 FFM tables [26, 208000, 16] f32 into one bf16
    row-major table G2 [208000, 512]: cols 0:416 = the 26 tables' rows for
    that index (feature f = 16*i + d), col 416 = fm1_emb, rest zero.
    1024B rows satisfy dma_gather's 256B-multiple elem/stride rule.
  - Gather via InstDMAGatherAnt (Q7 'mlp' library): 13 HBM gathers of 1024
    rows (2 fields x 512 batch) land g_all [128, 104rows, 512] (row 4j+s),
    then 26 SBUF-source TRANSPOSED dma_gathers (512 idxs each, <=512 per
    instr is a Q7 limit; HBM gathers cap at 1024) produce gT tiles
    [128, 4, 512] = K-chunks x batch -- the matmul rhs layout directly.
    No PE transposes, no PSUM->SBUF copies.
  - rd = relu(X_dense @ dense_W.T + b) is folded in TRANSPOSED space: per
    K-chunk a tiny PE matmul preT = dwr_chunk.T @ xdt into PSUM, then one
    DVE op rhs = relu(preT) + gT_chunk. g_all is never mutated, so fm2
    never races the fold.
  - fm2 via the i<j triangle on DVE straight out of g_all (one
    scalar_tensor_tensor per (s, j>=1) with custom APs), fm1 via 4 strided
    reduces of col 416.
  - h1.T accumulated on PE over 104 K=128 chunks (tail chunks K=32), W1
    pre-permuted/chunk-packed on host and streamed in 8 SBUF slabs.
  - BatchNorm batch stats all-reduced across the 8 cores (two tiny
    AllReduces) with a dummy warm-up AllReduce at t=0 to absorb the first
    collective's setup latency under the gather phase.
"""

import os
import sys

for _p in ("/opt/trn_rl_repo",):
    if _p not in sys.path and os.path.isdir(_p):
        sys.path.insert(0, _p)

import numpy as np
import ml_dtypes

from concourse import bass, mybir
import concourse.tile as tile
from concourse import library_config, library_overlay
from concourse.vector_clock import ScopedClock
from concourse.bass_utils import run_bass_kernel_spmd

BF16 = mybir.dt.bfloat16
F32 = mybir.dt.float32
I16 = mybir.dt.int16
AF = mybir.ActivationFunctionType
OP = mybir.AluOpType

N_CORES = int(os.environ.get("DFM_N_CORES", "8"))
F = 26
V_FIELD = 8000
V = F * V_FIELD            # 208000
D = 16
FD = F * D                 # 416
ELEM = 512                 # G2 row: 416 features + fm1 col (416) + pad
FM1_COL = 416
DNN_IN = F * F * D         # 10816
H1, H2 = 256, 128
BS = 4096
SHARD = BS // 8            # 512
NS = SHARD // 128          # 4
NDENSE = 13
EPS = 1e-5

NGRP = 13                  # field groups of 2 (HBM gather = 1024 idxs)
# Main-matmul K-chunk schedule: 3 full 128-row chunks per field, plus the
# 32-row tails packed 4-fields-per-chunk (partition-shifted fuse): 78+7=85.
CHUNKS2 = []               # ("full", j, c) | ("pack", t)
for _k in range(NGRP):
    for _jl in range(2):
        for _c in range(3):
            CHUNKS2.append(("full", 2 * _k + _jl, _c))
    if _k % 2 == 1:
        CHUNKS2.append(("pack", _k // 2))
CHUNKS2.append(("pack", 6))
NCHUNK = len(CHUNKS2)      # 85
SLAB = 13                  # w1 chunks per SBUF slab
NSLAB = (NCHUNK + SLAB - 1) // SLAB


def _chunk_k(c):
    return 128 if c < 3 else 32


def _install_drain_split():
    """This container's walrus rejects >1 sync-wait per TPB_CTRL instruction;
    split the Tile kernel-tail drain's waits onto single-wait NOPs."""
    if getattr(tile.TileContext, "_dfm_drain_patched", False):
        return

    def _split_drain_and_barrier(self, tick_clock, wait_clock):
        collector = self.nc.sync.nop(nofuse=True)
        wait_clock.add_sem_waits(
            collector.ins, ScopedClock({None: tick_clock.global_clock})
        )
        si = collector.ins.sync_info
        waits = list(si.on_wait) if si is not None else []
        if len(waits) > 1:
            si.on_wait = waits[:1]
            for i in range(1, len(waits)):
                extra = self.nc.sync.nop(nofuse=True)
                extra.ins.sync_info = mybir.SyncInfo(
                    on_wait=[waits[i]], on_update=[]
                )
        self.nc.sync.drain()
        self.nc.all_engine_barrier()
        assert self.sems is not None
        popped = self.nc._tile_sem_poison_stack.pop()
        assert popped is self._sem_poison
        self.nc.clear_and_free_semaphores(list(self.sems.allocated().values()))
        self.nc.all_engine_barrier()

    tile.TileContext._drain_and_barrier = _split_drain_and_barrier
    tile.TileContext._dfm_drain_patched = True


def _split_multiwaits(nc, max_waits=1):
    """This walrus build also rejects >1 sync-wait on regular engine
    instructions: hoist extra waits onto single-wait NOPs just before."""
    n_split = 0
    for fn in nc.m.functions:
        for bb in fn.blocks:
            new_insts = []
            for inst in bb.instructions:
                si = getattr(inst, "sync_info", None)
                waits = list(si.on_wait) if si is not None and si.on_wait else []
                if len(waits) > max_waits:
                    keep = waits[-max_waits:]
                    for k, w in enumerate(waits[:-max_waits]):
                        nop = mybir.InstNoOp(
                            name=f"{inst.name}_w{k}",
                            engine=inst.engine,
                            sync_info=mybir.SyncInfo(on_wait=[w], on_update=[]),
                            bass_nofuse=True,
                        )
                        new_insts.append(nop)
                    si.on_wait = keep
                    n_split += 1
                new_insts.append(inst)
            bb.instructions[:] = new_insts
    return n_split


def build_program():
    _install_drain_split()
    nc = bass.Bass(num_swdge_queues=2)

    g2_d = nc.declare_dram_parameter("g2", [V, ELEM], BF16, isOutput=False)
    idx_d = nc.declare_dram_parameter("idx", [128, NGRP * 64], I16, isOutput=False)
    idxt_d = nc.declare_dram_parameter("idxt", [128, 16], I16, isOutput=False)
    w1_d = nc.declare_dram_parameter("w1", [128, NCHUNK * H1], BF16, isOutput=False)
    dwr_d = nc.declare_dram_parameter("dwr", [NDENSE + 1, DNN_IN], BF16, isOutput=False)
    dwrt_d = nc.declare_dram_parameter("dwrt", [NDENSE + 1, 7 * 128], BF16, isOutput=False)
    xdt_d = nc.declare_dram_parameter("xdt", [NDENSE + 1, SHARD], BF16, isOutput=False)
    w2_d = nc.declare_dram_parameter("w2", [128, H1], BF16, isOutput=False)
    wout_d = nc.declare_dram_parameter("wout", [128, 1], BF16, isOutput=False)
    fm1w_d = nc.declare_dram_parameter("fm1w", [NDENSE, 1], BF16, isOutput=False)
    bn1g_d = nc.declare_dram_parameter("bn1g", [128, 2], F32, isOutput=False)
    bn1b_d = nc.declare_dram_parameter("bn1b", [128, 2], F32, isOutput=False)
    bn2g_d = nc.declare_dram_parameter("bn2g", [128, 1], F32, isOutput=False)
    bn2b_d = nc.declare_dram_parameter("bn2b", [128, 1], F32, isOutput=False)
    c0_d = nc.declare_dram_parameter("c0", [128, 1], F32, isOutput=False)
    out_d = nc.declare_dram_parameter("out", [SHARD, 1], F32, isOutput=True)

    with tile.TileContext(nc) as tc:
        with (
            tc.tile_pool(name="persist", bufs=1) as persist,
            tc.tile_pool(name="w1p", bufs=3) as w1p,
            tc.tile_pool(name="gtp", bufs=6) as gtp,
            tc.tile_pool(name="rhsp", bufs=4) as rhsp,
            tc.tile_pool(name="scrp", bufs=2) as scrp,
            tc.tile_pool(name="small", bufs=2) as small,
            tc.tile_pool(name="ps_h1", bufs=1, space="PSUM") as ps_h1,
            tc.tile_pool(name="ps_pre", bufs=2, space="PSUM") as ps_pre,
            tc.tile_pool(name="ps_small", bufs=1, space="PSUM") as ps_small,
            tc.tile_pool(name="dram", bufs=1, space="DRAM") as dram,
        ):
            nc.gpsimd.load_library(library_config.mlp)

            # ---- persistent loads ----
            idx = persist.tile([128, NGRP * 64], I16, tag="idx")
            nc.sync.dma_start(idx[:], idx_d[:])
            idxt = persist.tile([128, 16], I16, tag="idxt")
            nc.sync.dma_start(idxt[:], idxt_d[:])
            dwr = persist.tile([NDENSE + 1, DNN_IN], BF16, tag="dwr")
            nc.sync.dma_start(dwr[:], dwr_d[:])
            dwrt = persist.tile([NDENSE + 1, 7 * 128], BF16, tag="dwrt")
            nc.sync.dma_start(dwrt[:], dwrt_d[:])
            xdt = persist.tile([NDENSE + 1, SHARD], BF16, tag="xdt")
            nc.sync.dma_start(xdt[:], xdt_d[:])
            w2 = persist.tile([128, H1], BF16, tag="w2")
            nc.sync.dma_start(w2[:], w2_d[:])
            wout = persist.tile([128, 1], BF16, tag="wout")
            nc.sync.dma_start(wout[:], wout_d[:])
            fm1w = persist.tile([NDENSE, 1], BF16, tag="fm1w")
            nc.sync.dma_start(fm1w[:], fm1w_d[:])
            bn1g = persist.tile([128, 2], F32, tag="bn1g")
            nc.sync.dma_start(bn1g[:], bn1g_d[:])
            bn1b = persist.tile([128, 2], F32, tag="bn1b")
            nc.sync.dma_start(bn1b[:], bn1b_d[:])
            bn2g = persist.tile([128, 1], F32, tag="bn2g")
            nc.sync.dma_start(bn2g[:], bn2g_d[:])
            bn2b = persist.tile([128, 1], F32, tag="bn2b")
            nc.sync.dma_start(bn2b[:], bn2b_d[:])
            c0 = persist.tile([128, 1], F32, tag="c0")
            nc.sync.dma_start(c0[:], c0_d[:])

            warm_in = dram.tile([128, 1], F32, tag="warm_i")
            warm_out = dram.tile([128, 1], F32, tag="warm_o")
            nc.sync.dma_start(warm_in[:], c0[:])

            r1024 = nc.gpsimd.to_reg(1024)
            r128 = nc.gpsimd.to_reg(128)

            # ---- gathers + main matmul, pipelined per 2-field group ----
            g_all = persist.tile([128, 8 * NGRP, ELEM], BF16, tag="g_all")
            gap = g_all[:].rearrange("p a b -> p (a b)")
            part0 = list(g_all[:].ap[0])
            gbase = g_all[:].offset

            def hbm_gather(k):
                nc.gpsimd.dma_gather(
                    g_all[:, 8 * k:8 * (k + 1), :],
                    g2_d[16000 * k:16000 * (k + 1), :],
                    idx[:, 64 * k:64 * (k + 1)],
                    1024, r1024, ELEM,
                )

            h1_ps = [
                ps_h1.tile([128, SHARD], F32, tag=f"h1_{h}", name=f"h1_ps{h}")
                for h in range(2)
            ]
            fm1e = persist.tile([128, NS], F32, tag="fm1e")
            Spart = persist.tile([128, NS, F], F32, tag="Spart")

            hbm_gather(0)
            hbm_gather(1)
            # collective warm-up: absorbs first-CC setup under the gathers
            nc.gpsimd.collective_compute(
                "AllReduce", OP.add,
                replica_groups=[list(range(N_CORES))],
                ins=[warm_in.opt()], outs=[warm_out.opt()],
            )
            slabs = {}
            for sl in range(2):
                slabs[sl] = w1p.tile([128, SLAB * H1], BF16, tag="w1s",
                                     name=f"w1s{sl}")
                nc.sync.dma_start(
                    slabs[sl][:], w1_d[:, sl * SLAB * H1:(sl + 1) * SLAB * H1])

            gts = {}
            state = {"q": 0}

            def emit_chunk(ch):
                q = state["q"]
                sl, col = q // SLAB, (q % SLAB) * H1
                if col == 0 and sl >= 2:
                    wid = min(SLAB * H1, NCHUNK * H1 - sl * SLAB * H1)
                    slabs[sl] = w1p.tile([128, wid], BF16,
                                         tag="w1s", name=f"w1s{sl}")
                    nc.sync.dma_start(
                        slabs[sl][:],
                        w1_d[:, sl * SLAB * H1: sl * SLAB * H1 + wid])
                if ch[0] == "full":
                    _, j, c = ch
                    kc = 128
                else:
                    t = ch[1]
                    nf = min(4, F - 4 * t)
                    kc = 32 * nf
                pre = ps_pre.tile([128, SHARD], F32, tag="pre")
                if ch[0] == "full":
                    lhs_pre = dwr[:, j * FD + c * 128: j * FD + c * 128 + kc]
                else:
                    lhs_pre = dwrt[:, t * 128: t * 128 + kc]
                nc.tensor.matmul(
                    pre[0:kc, :], lhsT=lhs_pre, rhs=xdt[:, :],
                    start=True, stop=True,
                )
                rhs = rhsp.tile([128, SHARD], BF16, tag="rhs")
                if ch[0] == "full":
                    nc.vector.scalar_tensor_tensor(
                        out=rhs[0:kc, :], in0=pre[0:kc, :], scalar=0.0,
                        in1=gts[j][0:kc, :, c, :], op0=OP.max, op1=OP.add,
                    )
                else:
                    for u in range(nf):
                        nc.vector.scalar_tensor_tensor(
                            out=rhs[32 * u:32 * (u + 1), :],
                            in0=pre[32 * u:32 * (u + 1), :], scalar=0.0,
                            in1=gts[4 * t + u][0:32, :, 3, :],
                            op0=OP.max, op1=OP.add,
                        )
                for h in range(2):
                    nc.tensor.matmul(
                        h1_ps[h][:],
                        lhsT=slabs[sl][0:kc, col + h * 128: col + (h + 1) * 128],
                        rhs=rhs[0:kc, :],
                        start=(q == 0), stop=(q == NCHUNK - 1),
                    )
                state["q"] = q + 1

            for k in range(NGRP):
                if k + 2 < NGRP:
                    hbm_gather(k + 2)
                # fm2 for this group's two fields (reads g_all groups <= k)
                for jl in range(2):
                    j = 2 * k + jl
                    if j == 0:
                        continue
                    for s in range(NS):
                        in0 = bass.AP(
                            g_all[:].tensor, gbase + (4 * j + s) * ELEM,
                            [part0, [D, j], [1, D]],
                        )
                        in1 = bass.AP(
                            g_all[:].tensor, gbase + s * ELEM + D * j,
                            [part0, [4 * ELEM, j], [1, D]],
                        )
                        scr = scrp.tile([128, FD], BF16, tag="scr")
                        nc.vector.scalar_tensor_tensor(
                            out=scr[:, :j * D].rearrange("p (i d) -> p i d", d=D),
                            in0=in0, scalar=1.0, in1=in1,
                            op0=OP.mult, op1=OP.mult,
                            accum_out=Spart[:, s, j:j + 1],
                        )
                for jl in range(2):
                    j = 2 * k + jl
                    gt = gtp.tile([128, 16, 128], BF16, tag="gt",
                                  name=f"gt{j}")
                    nc.gpsimd.dma_gather(
                        gt[:],
                        g_all[:, 8 * k:8 * (k + 1), :].rearrange("p a b -> p (a b)"),
                        idxt[:, jl * 8:(jl + 1) * 8],
                        128, r128, 4 * ELEM,
                        transpose=True,
                        sbuf_tokens_per_rank=128,
                        sbuf_free_dim_per_rank=4 * ELEM * 2,
                        queue_num=1,
                    )
                    gts[j] = gt[:].rearrange("p (s c) b -> p s c b", s=NS)
                    for c in range(3):
                        emit_chunk(("full", j, c))
                if k % 2 == 1:
                    emit_chunk(("pack", k // 2))
                if k == NGRP - 1:
                    emit_chunk(("pack", 6))
                if k == 8:
                    # second warm-up keeps the CC stream hot for BN1's AR
                    nc.gpsimd.collective_compute(
                        "AllReduce", OP.add,
                        replica_groups=[list(range(N_CORES))],
                        ins=[warm_in.opt()], outs=[warm_out.opt()],
                    )

            # ---- fm1 + fm2 reduction ----
            S_acc = small.tile([128, NS], F32, tag="S_acc")
            for s in range(NS):
                fm1_ap = bass.AP(
                    g_all[:].tensor, gbase + s * ELEM + FM1_COL,
                    [part0, [4 * ELEM, F]],
                )
                nc.vector.tensor_reduce(
                    out=fm1e[:, s:s + 1], in_=fm1_ap,
                    axis=mybir.AxisListType.X, op=OP.add,
                )
                nc.vector.tensor_reduce(
                    out=S_acc[:, s:s + 1], in_=Spart[:, s, 1:F],
                    axis=mybir.AxisListType.X, op=OP.add,
                )

            # ---- BN1 stats + allreduce ----
            stats1 = small.tile([128, 4], F32, tag="stats1")
            sq_scr = persist.tile([128, SHARD], F32, tag="sq")
            for h in range(2):
                nc.vector.tensor_reduce(
                    out=stats1[:, h:h + 1], in_=h1_ps[h][:],
                    axis=mybir.AxisListType.X, op=OP.add,
                )
                nc.scalar.activation(
                    out=sq_scr[:], in_=h1_ps[h][:], func=AF.Square,
                    accum_out=stats1[:, 2 + h:3 + h],
                )
            b1_in = dram.tile([128, 4], F32, tag="b1i")
            b1_out = dram.tile([128, 4], F32, tag="b1o")
            nc.sync.dma_start(b1_in[:], stats1[:])
            nc.gpsimd.collective_compute(
                "AllReduce", OP.add,
                replica_groups=[list(range(N_CORES))],
                ins=[b1_in.opt()], outs=[b1_out.opt()],
            )
            stats1g = small.tile([128, 4], F32, tag="stats1g")
            nc.sync.dma_start(stats1g[:], b1_out[:])

            def bn_scale_bias(statsg, col_s, col_q, gamma, beta, ncols):
                mean = small.tile([128, ncols], F32, tag="bn_mean")
                var = small.tile([128, ncols], F32, tag="bn_var")
                scale = small.tile([128, ncols], F32, tag="bn_scale")
                bias = small.tile([128, ncols], F32, tag="bn_bias")
                tmp = small.tile([128, ncols], F32, tag="bn_tmp")
                nc.vector.tensor_scalar_mul(
                    mean[:], statsg[:, col_s:col_s + ncols], 1.0 / BS)
                nc.vector.tensor_scalar_mul(
                    var[:], statsg[:, col_q:col_q + ncols], 1.0 / BS)
                nc.vector.tensor_tensor(
                    out=tmp[:], in0=mean[:], in1=mean[:], op=OP.mult)
                nc.vector.tensor_tensor(
                    out=var[:], in0=var[:], in1=tmp[:], op=OP.subtract)
                nc.vector.tensor_scalar_add(var[:], var[:], EPS)
                nc.vector.reciprocal(tmp[:], var[:])
                nc.scalar.activation(out=tmp[:], in_=tmp[:], func=AF.Sqrt)
                nc.vector.tensor_tensor(
                    out=scale[:], in0=gamma[:], in1=tmp[:], op=OP.mult)
                nc.vector.tensor_tensor(
                    out=tmp[:], in0=mean[:], in1=scale[:], op=OP.mult)
                nc.vector.tensor_tensor(
                    out=bias[:], in0=beta[:], in1=tmp[:], op=OP.subtract)
                return scale, bias

            sc1, bi1 = bn_scale_bias(stats1g, 0, 2, bn1g, bn1b, 2)
            h1r = persist.tile([128, 2, SHARD], BF16, tag="h1r")
            for h in range(2):
                nc.scalar.activation(
                    out=h1r[:, h, :], in_=h1_ps[h][:], func=AF.Relu,
                    bias=bi1[:, h:h + 1], scale=sc1[:, h:h + 1],
                )

            # ---- layer 2 ----
            h2_ps = ps_small.tile([128, SHARD], F32, tag="h2")
            for h in range(2):
                nc.tensor.matmul(
                    h2_ps[:],
                    lhsT=w2[:, h * 128:(h + 1) * 128],
                    rhs=h1r[:, h, :],
                    start=(h == 0), stop=(h == 1),
                )
            stats2 = small.tile([128, 2], F32, tag="stats2")
            nc.vector.tensor_reduce(
                out=stats2[:, 0:1], in_=h2_ps[:],
                axis=mybir.AxisListType.X, op=OP.add,
            )
            nc.scalar.activation(
                out=sq_scr[:], in_=h2_ps[:], func=AF.Square,
                accum_out=stats2[:, 1:2],
            )
            b2_in = dram.tile([128, 2], F32, tag="b2i")
            b2_out = dram.tile([128, 2], F32, tag="b2o")
            nc.sync.dma_start(b2_in[:], stats2[:])
            nc.gpsimd.collective_compute(
                "AllReduce", OP.add,
                replica_groups=[list(range(N_CORES))],
                ins=[b2_in.opt()], outs=[b2_out.opt()],
            )
            stats2g = small.tile([128, 2], F32, tag="stats2g")
            nc.sync.dma_start(stats2g[:], b2_out[:])
            sc2, bi2 = bn_scale_bias(stats2g, 0, 1, bn2g, bn2b, 1)
            h2r = persist.tile([128, SHARD], BF16, tag="h2r")
            nc.scalar.activation(
                out=h2r[:], in_=h2_ps[:], func=AF.Relu,
                bias=bi2[:, 0:1], scale=sc2[:, 0:1],
            )

            # ---- heads ----
            head_ps = ps_small.tile([128, 2 * NS], F32, tag="heads")
            for s in range(NS):
                nc.tensor.matmul(
                    head_ps[:, s:s + 1],
                    lhsT=h2r[:, s * 128:(s + 1) * 128],
                    rhs=wout[:],
                    start=True, stop=True,
                )
                nc.tensor.matmul(
                    head_ps[:, NS + s:NS + s + 1],
                    lhsT=xdt[0:NDENSE, s * 128:(s + 1) * 128],
                    rhs=fm1w[:],
                    start=True, stop=True,
                )

            tot = small.tile([128, NS], F32, tag="tot")
            res = small.tile([128, NS], F32, tag="res")
            nc.vector.tensor_tensor(
                out=tot[:], in0=fm1e[:], in1=head_ps[:, 0:NS], op=OP.add)
            nc.vector.tensor_tensor(
                out=tot[:], in0=tot[:], in1=head_ps[:, NS:2 * NS], op=OP.add)
            nc.vector.tensor_tensor(
                out=tot[:], in0=tot[:], in1=S_acc[:], op=OP.add)
            nc.scalar.activation(
                out=res[:], in_=tot[:], func=AF.Sigmoid,
                bias=c0[:, 0:1], scale=1.0,
            )
            out_ap = out_d[:, :].rearrange("(s p) o -> p (s o)", p=128)
            nc.sync.dma_start(out_ap, res[:])

    library_overlay.lower_extended_insts(nc)
    _split_multiwaits(nc)
    return nc


_NC_CACHE = None


def _get_nc():
    global _NC_CACHE
    if _NC_CACHE is None:
        _NC_CACHE = build_program()
    return _NC_CACHE


def _wrap_idx16(vals):
    """[n] -> [128, n//16] int16: idx i at [i%16, i//16], replicated x8."""
    n = len(vals)
    a = np.asarray(vals, dtype=np.int16).reshape(n // 16, 16).T
    return np.tile(a, (8, 1))


def make_in_maps(X_sparse, X_dense, fm1_emb, bias, fm1_dense_W, fm1_dense_b,
                 emb_tables, dense_W, dense_b,
                 W1, b1, g1, beta1, W2, b2, g2, beta2, Wout, bout):
    bf16 = ml_dtypes.bfloat16
    f32 = np.float32

    g2t = np.zeros((V, ELEM), dtype=bf16)
    g2t[:, 0:FD] = (
        np.ascontiguousarray(emb_tables.transpose(1, 0, 2)).reshape(V, FD)
        .astype(bf16)
    )
    g2t[:, FM1_COL] = fm1_emb[:, 0].astype(bf16)

    # W1 permuted to (j, f=16i+d) rows, packed into 104 K-chunks of 128.
    W1p = np.ascontiguousarray(
        W1.reshape(H1, F, F, D).transpose(2, 1, 3, 0)
    ).reshape(F, FD, H1)
    w1k = np.zeros((NCHUNK, 128, H1), dtype=f32)
    for q, ch in enumerate(CHUNKS2):
        if ch[0] == "full":
            _, j, c = ch
            w1k[q] = W1p[j, c * 128:(c + 1) * 128]
        else:
            t = ch[1]
            for u in range(min(4, F - 4 * t)):
                w1k[q, 32 * u:32 * u + 32] = W1p[4 * t + u, 384:416]
    w1h = np.ascontiguousarray(w1k.transpose(1, 0, 2)).reshape(
        128, NCHUNK * H1).astype(bf16)

    dWr = np.ascontiguousarray(
        dense_W.reshape(F, F, D, NDENSE).transpose(1, 0, 2, 3)
    ).reshape(DNN_IN, NDENSE)
    dwrh = np.zeros((NDENSE + 1, DNN_IN), dtype=bf16)
    dwrh[0:NDENSE] = dWr.T.astype(bf16)
    dwrh[NDENSE] = np.ascontiguousarray(
        dense_b.reshape(F, F, D).transpose(1, 0, 2)
    ).reshape(DNN_IN).astype(bf16)

    dwrth = np.zeros((NDENSE + 1, 7 * 128), dtype=bf16)
    for t in range(7):
        for u in range(min(4, F - 4 * t)):
            dwrth[:, t * 128 + 32 * u: t * 128 + 32 * u + 32] = \
                dwrh[:, (4 * t + u) * FD + 384: (4 * t + u) * FD + 416]

    w2h = np.ascontiguousarray(
        W2.T.reshape(2, 128, H2).transpose(1, 0, 2)
    ).reshape(128, H1).astype(bf16)
    wouth = Wout.reshape(H2, 1).astype(bf16) if Wout.shape == (H2, 1) else \
        Wout.T.astype(bf16)
    fm1wh = fm1_dense_W.T.astype(bf16)  # [13, 1]

    bn1gh = np.ascontiguousarray(g1.reshape(2, 128).T).astype(f32)
    bn1bh = np.ascontiguousarray(beta1.reshape(2, 128).T).astype(f32)
    bn2gh = g2.reshape(128, 1).astype(f32)
    bn2bh = beta2.reshape(128, 1).astype(f32)
    c0h = np.full((128, 1),
                  float(bias[0]) + float(fm1_dense_b[0]) + float(bout[0]),
                  dtype=f32)

    idxt_h = np.concatenate(
        [_wrap_idx16(np.arange(128) + 128 * jl) for jl in range(2)], axis=1)

    in_maps = []
    for core in range(N_CORES):
        Xl = X_sparse[core * SHARD:(core + 1) * SHARD]   # [512, 26] local
        idx_h = np.zeros((128, NGRP * 64), dtype=np.int16)
        for k in range(NGRP):
            vals = np.empty(1024, dtype=np.int64)
            for jl in range(2):
                vals[jl * 512:(jl + 1) * 512] = Xl[:, 2 * k + jl] + jl * V_FIELD
            idx_h[:, 64 * k:64 * (k + 1)] = _wrap_idx16(vals)
        xdt_c = np.ones((NDENSE + 1, SHARD), dtype=bf16)
        xdt_c[0:NDENSE] = X_dense[core * SHARD:(core + 1) * SHARD].T.astype(bf16)
        in_maps.append({
            "g2": g2t, "idx": idx_h, "idxt": idxt_h, "w1": w1h, "dwr": dwrh, "dwrt": dwrth,
            "xdt": xdt_c, "w2": w2h, "wout": wouth, "fm1w": fm1wh,
            "bn1g": bn1gh, "bn1b": bn1bh, "bn2g": bn2gh, "bn2b": bn2bh,
            "c0": c0h,
        })
    return in_maps


def kernel(**inputs):
    nc = _get_nc()
    in_maps = make_in_maps(**{k: np.asarray(v) for k, v in inputs.items()})
    res = run_bass_kernel_spmd(
        nc, in_maps, core_ids=list(range(N_CORES)),
        trace=bool(int(os.environ.get("DFM_TRACE", "0"))),
    )
    out = np.concatenate([res.results[c]["out"] for c in range(N_CORES)], axis=0)
    kernel.last_results = res
    return out.astype(np.float32)
